# revision 29
# baseline (speedup 1.0000x reference)
"""BetweennessRoPE Trainium2 kernel — fixed-table fp16 formulation.

Math (why no betweenness is computed on device):
  score = relu(1 - (path-direct)/max(direct,1e-6)) lies in [0,1] by the
  triangle inequality, so between in [0, 1/2046] and
  pos_adj = (between-0.5)*0.1 in [-0.05, -0.05+4.888e-5].  Hence for
  every position frac = 0.95 + delta with |delta| <= ~1.1e-4 (including
  the fp32 rounding of fl(s + pos_adj) at s ~ 2048).  The interpolated
  tables therefore differ from fixed-f tables
      C[s] = (1-f)*fcos[s-1] + f*fcos[s],  f = 0.95 + 0.05/2046
  by <= ~1.1e-4 * |fcos[s]-fcos[s-1]|, giving output error ~2e-4 of the
  output scale — far below the 2e-2 gate.  s=0 is exact (clip pins
  adj_pos to 0 and C[0]=fcos[0]).  So the kernel is a pure elementwise
  rotation with per-(s,k) constants:
      oe = xe*cc - xo*ss ;  oo = xo*cc + xe*ss.

Numerics: fp16 x / tables / products / outputs (DVE computes fp32
internally, rounds once on write) add ~1.5e-3 relative noise — still
~10x under the gate — and halve both DMA traffic and DVE cycles
(2x_1P packed mode needs 16-bit dense operands).

Layout: host de-interleaves even/odd features and converts to fp16.
Per slice [128, 2048]: partition p, col (e, t, k), s = 128t + p,
d = 2k + e.  Table t1 = [cc|-ss] is DMA'd (halves interleaved with the
first x slice's halves so DVE starts after only 0.5 MiB of DMA);
t2 = [ss|cc] is derived on-device with two cheap 4x-mode DVE ops.
Per slice, 3 VectorE TT ops at the fp16 2x packed rate (FD=2048,
~1.22 us each):  pq = [x*t1 | x*t2], then one fused add over 4D views
folds the e-halves into og = [oe | oo].  The last slice splits its add
and store in half so the out stream drains during the final add.

Engine notes (measured): GpSimd stays idle on purpose — concurrent Q7
TT ops contend on the shared DVE/POOL SBUF port (2.4-2.9x DVE
slowdown in a fp32 test).  Strided-dest or stride-0-broadcast DVE ops
run ~20% slow; plain-2D dests with strided sources run at full rate.
Wider (pairwise) ops save nothing — DVE back-to-back op overhead is
only ~80 ns.  Input + table ride the SyncE DMA queue (8 x-buffers so
all loads issue up front at ~365 GB/s); output stores ride the ScalarE
queue.  Typical HW exec ~45-48 us/core: ~8.5 us fixed NEFF preamble +
~2 us first data + ~27.3 us saturated DVE + store tail/postamble.
"""

import numpy as np

B, S, H, D = 4, 2048, 16, 128
N = B * H
NCORES = 8
NPC = N // NCORES    # 8 slices per core
NT = S // 128        # 16
K2 = D // 2          # 64
HK = S // 2          # 1024 (cols per e-half)

_cache = {}


def _make_tables():
    base = (1.0 / (10000.0 ** (np.arange(0, D, 2, dtype=np.float32)
                               / np.float32(D)))).astype(np.float32)
    freqs = (np.arange(S, dtype=np.float32)[:, None]
             * base[None, :]).astype(np.float32)
    fcos = np.cos(freqs).astype(np.float32)
    fsin = np.sin(freqs).astype(np.float32)
    lo = np.maximum(np.arange(S) - 1, 0)
    f = 0.95 + 0.05 / 2046.0
    cc = ((1.0 - f) * fcos[lo].astype(np.float64)
          + f * fcos.astype(np.float64))
    ss = ((1.0 - f) * fsin[lo].astype(np.float64)
          + f * fsin.astype(np.float64))

    def blk(t):  # [S, 64] -> [128, NT*64], col (t, k)
        return t.reshape(NT, 128, K2).transpose(1, 0, 2).reshape(128, HK)

    t1 = np.concatenate([blk(cc), blk(-ss)], axis=1)       # [128, 2048]
    return np.ascontiguousarray(t1).astype(np.float16)


def _build_nc():
    import concourse.bacc as bacc
    import concourse.mybir as mybir
    from concourse.tile import TileContext

    f16 = mybir.dt.float16

    nc = bacc.Bacc()
    XC = nc.dram_tensor("XC", [NPC, 128, S], f16, kind="ExternalInput")
    OUT = nc.dram_tensor("OUT", [NPC, 128, S], f16, kind="ExternalOutput")
    CB = nc.dram_tensor("CB", [128, S], f16, kind="ExternalInput")

    with TileContext(nc) as tc:
        with (
            tc.tile_pool(name="const", bufs=1) as cpool,
            tc.tile_pool(name="xbuf", bufs=4) as xpool,
            tc.tile_pool(name="obuf", bufs=4) as opool,
            tc.tile_pool(name="pq", bufs=2) as wpool,
        ):
            tb = cpool.tile([128, 2 * S], f16, tag="tb", name="tb")
            xts = {}

            def load_single(n):
                xt = xpool.tile([128, S], f16, tag="x1", bufs=8,
                                name=f"x{n}")
                nc.sync.dma_start(xt[:, :], XC[n])
                xts[n] = xt
                return xt

            # startup: interleave table halves with slice-0 halves so the
            # first mul only waits for 0.5 MiB of DMA
            x0 = xpool.tile([128, S], f16, tag="x1", bufs=8, name="x0")
            nc.sync.dma_start(tb[:, 0:HK], CB[:, 0:HK])          # cc
            nc.sync.dma_start(x0[:, 0:HK], XC[0][:, 0:HK])       # xe
            nc.sync.dma_start(tb[:, HK:S], CB[:, HK:S])          # -ss
            nc.sync.dma_start(x0[:, HK:S], XC[0][:, HK:S])       # xo
            xts[0] = x0
            load_single(1)
            # t2 = [ss | cc] derived from t1 = [cc | -ss]
            nc.vector.tensor_copy(tb[:, S + HK:2 * S], tb[:, 0:HK])
            nc.vector.tensor_scalar_mul(tb[:, S:S + HK], tb[:, HK:S], -1.0)

            def compute_single(n, split_mul1=False, split_tail=False):
                xt = xts[n]
                pq = wpool.tile([128, 2 * S], f16, tag="PQ1", bufs=4,
                                name=f"PQ{n}")
                og = opool.tile([128, S], f16, tag="o1", bufs=6,
                                name=f"o{n}")
                if split_mul1:
                    nc.vector.tensor_mul(pq[:, 0:HK], xt[:, 0:HK],
                                         tb[:, 0:HK])
                    nc.vector.tensor_mul(pq[:, HK:S], xt[:, HK:S],
                                         tb[:, HK:S])
                else:
                    nc.vector.tensor_mul(pq[:, 0:S], xt[:, :], tb[:, 0:S])
                nc.vector.tensor_mul(pq[:, S:2 * S], xt[:, :], tb[:, S:2 * S])
                avn = pq[:, :].rearrange("p (m e c) -> p m e c", m=2, e=2)
                ovn = og[:, :].rearrange("p (m c) -> p m c", m=2)
                if split_tail:
                    nc.vector.tensor_add(ovn[:, 0, :], avn[:, 0, 0, :],
                                         avn[:, 0, 1, :])
                    nc.scalar.dma_start(OUT[n][:, 0:HK], og[:, 0:HK])
                    nc.vector.tensor_add(ovn[:, 1, :], avn[:, 1, 0, :],
                                         avn[:, 1, 1, :])
                    nc.sync.dma_start(OUT[n][:, HK:S], og[:, HK:S])
                else:
                    nc.vector.tensor_add(ovn[:, :, :], avn[:, :, 0, :],
                                         avn[:, :, 1, :])
                    nc.scalar.dma_start(OUT[n], og[:, :])

            compute_single(0, split_mul1=True)
            compute_single(1)
            for n in range(2, NPC - 1):
                load_single(n)
                compute_single(n)
            load_single(NPC - 1)
            compute_single(NPC - 1, split_tail=True)
    nc.compile()
    return nc


def _get_built():
    if "nc" not in _cache:
        _cache["nc"] = _build_nc()
    return _cache["nc"]


def kernel(x, W, b):
    from concourse.bass_utils import run_bass_kernel_spmd

    assert x.shape == (B, S, H, D)
    xc = np.transpose(np.asarray(x, dtype=np.float32),
                      (0, 2, 1, 3)).reshape(N, S, D)
    # col (e, t, k) <- xc[n, 128t+p, 2k+e], fp16
    xs = np.ascontiguousarray(
        xc.reshape(N, NT, 128, K2, 2).transpose(0, 2, 4, 1, 3)
        .reshape(N, 128, S)).astype(np.float16)
    if "cb" not in _cache:
        _cache["cb"] = _make_tables()
    cbb = _cache["cb"]

    nc = _get_built()
    in_maps = []
    for c in range(NCORES):
        in_maps.append({
            "XC": np.ascontiguousarray(xs[NPC * c:NPC * (c + 1)]),
            "CB": cbb,
        })
    res = run_bass_kernel_spmd(nc, in_maps, core_ids=list(range(NCORES)))
    if res.exec_time_ns is not None:
        print(f"HW exec time: {res.exec_time_ns} ns")
    outs = np.concatenate([res.results[c]["OUT"] for c in range(NCORES)],
                          axis=0)                   # [N, 128, S]
    # og col = (m, t, k): s = 128t + p, d = 2k + m
    full = (outs.reshape(N, 128, 2, NT, K2).transpose(0, 3, 1, 4, 2)
            .reshape(N, S, D).astype(np.float32))
    full = full.reshape(B, H, S, D).transpose(0, 2, 1, 3)
    return np.ascontiguousarray(full)


# revision 30
# speedup vs baseline: 1.0318x; 1.0318x over previous
"""BetweennessRoPE Trainium2 kernel — fixed-table fp16 formulation.

Math (why no betweenness is computed on device):
  score = relu(1 - (path-direct)/max(direct,1e-6)) lies in [0,1] by the
  triangle inequality, so between in [0, 1/2046] and
  pos_adj = (between-0.5)*0.1 in [-0.05, -0.05+4.888e-5].  Hence for
  every position frac = 0.95 + delta with |delta| <= ~1.1e-4 (including
  the fp32 rounding of fl(s + pos_adj) at s ~ 2048).  The interpolated
  tables therefore differ from fixed-f tables
      C[s] = (1-f)*fcos[s-1] + f*fcos[s],  f = 0.95 + 0.05/2046
  by <= ~1.1e-4 * |fcos[s]-fcos[s-1]|, giving output error ~2e-4 of the
  output scale — far below the 2e-2 gate.  s=0 is exact (clip pins
  adj_pos to 0 and C[0]=fcos[0]).  So the kernel is a pure elementwise
  rotation with per-(s,k) constants:
      oe = xe*cc - xo*ss ;  oo = xo*cc + xe*ss.

Numerics: fp16 x / tables / products / outputs (DVE computes fp32
internally, rounds once on write) add ~1.5e-3 relative noise — still
~10x under the gate — and halve both DMA traffic and DVE cycles
(2x_1P packed mode needs 16-bit dense operands).

Layout: host de-interleaves even/odd features and converts to fp16.
Per slice [128, 2048]: partition p, col (e, t, k), s = 128t + p,
d = 2k + e.  Table t1 = [cc|-ss] is DMA'd (halves interleaved with the
first x slice's halves so DVE starts after only 0.5 MiB of DMA);
t2 = [ss|cc] is derived on-device with two cheap 4x-mode DVE ops.
Per slice, 3 VectorE TT ops at the fp16 2x packed rate (FD=2048,
~1.22 us each):  pq = [x*t1 | x*t2], then one fused add over 4D views
folds the e-halves into og = [oe | oo].  The last slice splits its add
and store in half so the out stream drains during the final add.

Engine notes (measured): GpSimd stays idle on purpose — concurrent Q7
TT ops contend on the shared DVE/POOL SBUF port (2.4-2.9x DVE
slowdown in a fp32 test).  Strided-dest or stride-0-broadcast DVE ops
run ~20% slow; plain-2D dests with strided sources run at full rate.
Wider (pairwise) ops save nothing — DVE back-to-back op overhead is
only ~80 ns.  Input + table ride the SyncE DMA queue (8 x-buffers so
all loads issue up front at ~365 GB/s); output stores ride the ScalarE
queue.  Typical HW exec ~45-48 us/core: ~8.5 us fixed NEFF preamble +
~2 us first data + ~27.3 us saturated DVE + store tail/postamble.
"""

import numpy as np

B, S, H, D = 4, 2048, 16, 128
N = B * H
NCORES = 8
NPC = N // NCORES    # 8 slices per core
NT = S // 128        # 16
K2 = D // 2          # 64
HK = S // 2          # 1024 (cols per e-half)

_cache = {}


def _make_tables():
    base = (1.0 / (10000.0 ** (np.arange(0, D, 2, dtype=np.float32)
                               / np.float32(D)))).astype(np.float32)
    freqs = (np.arange(S, dtype=np.float32)[:, None]
             * base[None, :]).astype(np.float32)
    fcos = np.cos(freqs).astype(np.float32)
    fsin = np.sin(freqs).astype(np.float32)
    lo = np.maximum(np.arange(S) - 1, 0)
    f = 0.95 + 0.05 / 2046.0
    cc = ((1.0 - f) * fcos[lo].astype(np.float64)
          + f * fcos.astype(np.float64))
    ss = ((1.0 - f) * fsin[lo].astype(np.float64)
          + f * fsin.astype(np.float64))

    def blk(t):  # [S, 64] -> [128, NT*64], col (t, k)
        return t.reshape(NT, 128, K2).transpose(1, 0, 2).reshape(128, HK)

    t1 = np.concatenate([blk(cc), blk(-ss)], axis=1)       # [128, 2048]
    return np.ascontiguousarray(t1).astype(np.float16)


def _build_nc():
    import concourse.bacc as bacc
    import concourse.mybir as mybir
    from concourse.tile import TileContext

    f16 = mybir.dt.float16

    nc = bacc.Bacc()
    XC = nc.dram_tensor("XC", [NPC, 128, S], f16, kind="ExternalInput")
    OUT = nc.dram_tensor("OUT", [NPC, 128, S], f16, kind="ExternalOutput")
    CB = nc.dram_tensor("CB", [128, S], f16, kind="ExternalInput")

    with TileContext(nc) as tc:
        with (
            tc.tile_pool(name="const", bufs=1) as cpool,
            tc.tile_pool(name="xbuf", bufs=4) as xpool,
            tc.tile_pool(name="obuf", bufs=4) as opool,
            tc.tile_pool(name="pq", bufs=2) as wpool,
        ):
            tb = cpool.tile([128, 2 * S], f16, tag="tb", name="tb")
            xts = {}

            def load_single(n):
                xt = xpool.tile([128, S], f16, tag="x1", bufs=8,
                                name=f"x{n}")
                nc.sync.dma_start(xt[:, :], XC[n])
                xts[n] = xt
                return xt

            # startup: interleave table halves with slice-0 halves so the
            # first mul only waits for 0.5 MiB of DMA
            x0 = xpool.tile([128, S], f16, tag="x1", bufs=8, name="x0")
            nc.sync.dma_start(tb[:, 0:HK], CB[:, 0:HK])          # cc
            nc.sync.dma_start(x0[:, 0:HK], XC[0][:, 0:HK])       # xe
            nc.sync.dma_start(tb[:, HK:S], CB[:, HK:S])          # -ss
            nc.sync.dma_start(x0[:, HK:S], XC[0][:, HK:S])       # xo
            xts[0] = x0
            load_single(1)
            # t2 = [ss | cc] derived from t1 = [cc | -ss] on the otherwise
            # idle ScalarE, keeping the saturated DVE stream clear
            nc.scalar.copy(tb[:, S + HK:2 * S], tb[:, 0:HK])
            nc.scalar.mul(tb[:, S:S + HK], tb[:, HK:S], -1.0)

            def compute_single(n, split_mul1=False, split_tail=False):
                xt = xts[n]
                pq = wpool.tile([128, 2 * S], f16, tag="PQ1", bufs=4,
                                name=f"PQ{n}")
                og = opool.tile([128, S], f16, tag="o1", bufs=6,
                                name=f"o{n}")
                if split_mul1:
                    nc.vector.tensor_mul(pq[:, 0:HK], xt[:, 0:HK],
                                         tb[:, 0:HK])
                    nc.vector.tensor_mul(pq[:, HK:S], xt[:, HK:S],
                                         tb[:, HK:S])
                else:
                    nc.vector.tensor_mul(pq[:, 0:S], xt[:, :], tb[:, 0:S])
                nc.vector.tensor_mul(pq[:, S:2 * S], xt[:, :], tb[:, S:2 * S])
                avn = pq[:, :].rearrange("p (m e c) -> p m e c", m=2, e=2)
                ovn = og[:, :].rearrange("p (m c) -> p m c", m=2)
                if split_tail:
                    nc.vector.tensor_add(ovn[:, 0, :], avn[:, 0, 0, :],
                                         avn[:, 0, 1, :])
                    nc.scalar.dma_start(OUT[n][:, 0:HK], og[:, 0:HK])
                    nc.vector.tensor_add(ovn[:, 1, :], avn[:, 1, 0, :],
                                         avn[:, 1, 1, :])
                    nc.sync.dma_start(OUT[n][:, HK:S], og[:, HK:S])
                else:
                    nc.vector.tensor_add(ovn[:, :, :], avn[:, :, 0, :],
                                         avn[:, :, 1, :])
                    nc.scalar.dma_start(OUT[n], og[:, :])

            compute_single(0, split_mul1=True)
            compute_single(1)
            for n in range(2, NPC - 1):
                load_single(n)
                compute_single(n)
            load_single(NPC - 1)
            compute_single(NPC - 1, split_tail=True)
    nc.compile()
    return nc


def _get_built():
    if "nc" not in _cache:
        _cache["nc"] = _build_nc()
    return _cache["nc"]


def kernel(x, W, b):
    from concourse.bass_utils import run_bass_kernel_spmd

    assert x.shape == (B, S, H, D)
    xc = np.transpose(np.asarray(x, dtype=np.float32),
                      (0, 2, 1, 3)).reshape(N, S, D)
    # col (e, t, k) <- xc[n, 128t+p, 2k+e], fp16
    xs = np.ascontiguousarray(
        xc.reshape(N, NT, 128, K2, 2).transpose(0, 2, 4, 1, 3)
        .reshape(N, 128, S)).astype(np.float16)
    if "cb" not in _cache:
        _cache["cb"] = _make_tables()
    cbb = _cache["cb"]

    nc = _get_built()
    in_maps = []
    for c in range(NCORES):
        in_maps.append({
            "XC": np.ascontiguousarray(xs[NPC * c:NPC * (c + 1)]),
            "CB": cbb,
        })
    res = run_bass_kernel_spmd(nc, in_maps, core_ids=list(range(NCORES)))
    if res.exec_time_ns is not None:
        print(f"HW exec time: {res.exec_time_ns} ns")
    outs = np.concatenate([res.results[c]["OUT"] for c in range(NCORES)],
                          axis=0)                   # [N, 128, S]
    # og col = (m, t, k): s = 128t + p, d = 2k + m
    full = (outs.reshape(N, 128, 2, NT, K2).transpose(0, 3, 1, 4, 2)
            .reshape(N, S, D).astype(np.float32))
    full = full.reshape(B, H, S, D).transpose(0, 2, 1, 3)
    return np.ascontiguousarray(full)


# revision 31
# speedup vs baseline: 1.1406x; 1.1055x over previous
"""BetweennessRoPE Trainium2 kernel — fixed-table fp16 formulation.

Math (why no betweenness is computed on device):
  score = relu(1 - (path-direct)/max(direct,1e-6)) lies in [0,1] by the
  triangle inequality, so between in [0, 1/2046] and
  pos_adj = (between-0.5)*0.1 in [-0.05, -0.05+4.888e-5].  Hence for
  every position frac = 0.95 + delta with |delta| <= ~1.1e-4 (including
  the fp32 rounding of fl(s + pos_adj) at s ~ 2048).  The interpolated
  tables therefore differ from fixed-f tables
      C[s] = (1-f)*fcos[s-1] + f*fcos[s],  f = 0.95 + 0.05/2046
  by <= ~1.1e-4 * |fcos[s]-fcos[s-1]|, giving output error ~2e-4 of the
  output scale — far below the 2e-2 gate.  s=0 is exact (clip pins
  adj_pos to 0 and C[0]=fcos[0]).  So the kernel is a pure elementwise
  rotation with per-(s,k) constants:
      oe = xe*cc - xo*ss ;  oo = xo*cc + xe*ss.

Numerics: fp16 x / tables / products / outputs (DVE computes fp32
internally, rounds once on write) add ~1.5e-3 relative noise — still
~10x under the gate — and halve both DMA traffic and DVE cycles
(2x_1P packed mode needs 16-bit dense operands).

Layout: host de-interleaves even/odd features and converts to fp16.
Per slice [128, 2048]: partition p, col (e, t, k), s = 128t + p,
d = 2k + e.  Table t1 = [cc|-ss] is DMA'd (halves interleaved with the
first x slice's halves so DVE starts after only 0.5 MiB of DMA);
t2 = [ss|cc] is derived on-device with two cheap 4x-mode DVE ops.
Per slice, 3 VectorE TT ops at the fp16 2x packed rate (FD=2048,
~1.22 us each):  pq = [x*t1 | x*t2], then one fused add over 4D views
folds the e-halves into og = [oe | oo].  The last slice splits its add
and store in half so the out stream drains during the final add.

Engine notes (measured): GpSimd stays idle on purpose — concurrent Q7
TT ops contend on the shared DVE/POOL SBUF port (2.4-2.9x DVE
slowdown in a fp32 test).  Strided-dest or stride-0-broadcast DVE ops
run ~20% slow; plain-2D dests with strided sources run at full rate.
Wider (pairwise) ops save nothing — DVE back-to-back op overhead is
only ~80 ns.  Input + table ride the SyncE DMA queue (8 x-buffers so
all loads issue up front at ~365 GB/s); output stores ride the ScalarE
queue.  Typical HW exec ~45-48 us/core: ~8.5 us fixed NEFF preamble +
~2 us first data + ~27.3 us saturated DVE + store tail/postamble.
"""

import numpy as np

B, S, H, D = 4, 2048, 16, 128
N = B * H
NCORES = 8
NPC = N // NCORES    # 8 slices per core
NT = S // 128        # 16
K2 = D // 2          # 64
HK = S // 2          # 1024 (cols per e-half)

_cache = {}


def _make_tables():
    base = (1.0 / (10000.0 ** (np.arange(0, D, 2, dtype=np.float32)
                               / np.float32(D)))).astype(np.float32)
    freqs = (np.arange(S, dtype=np.float32)[:, None]
             * base[None, :]).astype(np.float32)
    fcos = np.cos(freqs).astype(np.float32)
    fsin = np.sin(freqs).astype(np.float32)
    lo = np.maximum(np.arange(S) - 1, 0)
    f = 0.95 + 0.05 / 2046.0
    cc = ((1.0 - f) * fcos[lo].astype(np.float64)
          + f * fcos.astype(np.float64))
    ss = ((1.0 - f) * fsin[lo].astype(np.float64)
          + f * fsin.astype(np.float64))

    def blk(t):  # [S, 64] -> [128, NT*64], col (t, k)
        return t.reshape(NT, 128, K2).transpose(1, 0, 2).reshape(128, HK)

    t1 = np.concatenate([blk(cc), blk(-ss)], axis=1)       # [128, 2048]
    return np.ascontiguousarray(t1).astype(np.float16)


def _build_nc():
    import concourse.bacc as bacc
    import concourse.mybir as mybir
    from concourse.tile import TileContext

    f16 = mybir.dt.float16

    nc = bacc.Bacc()
    XC = nc.dram_tensor("XC", [NPC, 128, S], f16, kind="ExternalInput")
    OUT = nc.dram_tensor("OUT", [NPC, 128, S], f16, kind="ExternalOutput")
    CB = nc.dram_tensor("CB", [128, S], f16, kind="ExternalInput")

    with TileContext(nc) as tc:
        with (
            tc.tile_pool(name="const", bufs=1) as cpool,
            tc.tile_pool(name="xbuf", bufs=4) as xpool,
            tc.tile_pool(name="obuf", bufs=4) as opool,
            tc.tile_pool(name="pq", bufs=2) as wpool,
        ):
            tb = cpool.tile([128, 2 * S], f16, tag="tb", name="tb")
            xts = {}

            def load_single(n):
                xt = xpool.tile([128, S], f16, tag="x1", bufs=8,
                                name=f"x{n}")
                nc.sync.dma_start(xt[:, :], XC[n])
                xts[n] = xt
                return xt

            # startup: interleave table halves with slice-0 halves so the
            # first mul only waits for 0.5 MiB of DMA
            x0 = xpool.tile([128, S], f16, tag="x1", bufs=8, name="x0")
            nc.sync.dma_start(tb[:, 0:HK], CB[:, 0:HK])          # cc
            nc.sync.dma_start(x0[:, 0:HK], XC[0][:, 0:HK])       # xe
            nc.sync.dma_start(tb[:, HK:S], CB[:, HK:S])          # -ss
            nc.sync.dma_start(x0[:, HK:S], XC[0][:, HK:S])       # xo
            xts[0] = x0
            load_single(1)
            # t2 = [ss | cc] derived from t1 = [cc | -ss] on the otherwise
            # idle ScalarE, keeping the saturated DVE stream clear
            nc.scalar.copy(tb[:, S + HK:2 * S], tb[:, 0:HK])
            nc.scalar.mul(tb[:, S:S + HK], tb[:, HK:S], -1.0)

            def compute_single(n, split_mul1=False, split_tail=False):
                xt = xts[n]
                pq = wpool.tile([128, 2 * S], f16, tag="PQ1", bufs=4,
                                name=f"PQ{n}")
                og = opool.tile([128, S], f16, tag="o1", bufs=6,
                                name=f"o{n}")
                if split_mul1:
                    nc.vector.tensor_mul(pq[:, 0:HK], xt[:, 0:HK],
                                         tb[:, 0:HK])
                    nc.vector.tensor_mul(pq[:, HK:S], xt[:, HK:S],
                                         tb[:, HK:S])
                else:
                    nc.vector.tensor_mul(pq[:, 0:S], xt[:, :], tb[:, 0:S])
                nc.vector.tensor_mul(pq[:, S:2 * S], xt[:, :], tb[:, S:2 * S])
                avn = pq[:, :].rearrange("p (m e c) -> p m e c", m=2, e=2)
                ovn = og[:, :].rearrange("p (m c) -> p m c", m=2)
                if split_tail:
                    nc.vector.tensor_add(ovn[:, 0, :], avn[:, 0, 0, :],
                                         avn[:, 0, 1, :])
                    nc.scalar.dma_start(OUT[n][:, 0:HK], og[:, 0:HK])
                    nc.vector.tensor_add(ovn[:, 1, :], avn[:, 1, 0, :],
                                         avn[:, 1, 1, :])
                    nc.sync.dma_start(OUT[n][:, HK:S], og[:, HK:S])
                else:
                    nc.vector.tensor_add(ovn[:, :, :], avn[:, :, 0, :],
                                         avn[:, :, 1, :])
                    # slices 4-6 store via SyncE (idle after the loads) so
                    # both queues are warm when the final slice drains
                    eng = nc.sync if n >= 4 else nc.scalar
                    eng.dma_start(OUT[n], og[:, :])

            compute_single(0, split_mul1=True)
            compute_single(1)
            for n in range(2, NPC - 1):
                load_single(n)
                compute_single(n)
            load_single(NPC - 1)
            compute_single(NPC - 1, split_tail=True)
    nc.compile()
    return nc


def _get_built():
    if "nc" not in _cache:
        _cache["nc"] = _build_nc()
    return _cache["nc"]


def kernel(x, W, b):
    from concourse.bass_utils import run_bass_kernel_spmd

    assert x.shape == (B, S, H, D)
    xc = np.transpose(np.asarray(x, dtype=np.float32),
                      (0, 2, 1, 3)).reshape(N, S, D)
    # col (e, t, k) <- xc[n, 128t+p, 2k+e], fp16
    xs = np.ascontiguousarray(
        xc.reshape(N, NT, 128, K2, 2).transpose(0, 2, 4, 1, 3)
        .reshape(N, 128, S)).astype(np.float16)
    if "cb" not in _cache:
        _cache["cb"] = _make_tables()
    cbb = _cache["cb"]

    nc = _get_built()
    in_maps = []
    for c in range(NCORES):
        in_maps.append({
            "XC": np.ascontiguousarray(xs[NPC * c:NPC * (c + 1)]),
            "CB": cbb,
        })
    res = run_bass_kernel_spmd(nc, in_maps, core_ids=list(range(NCORES)))
    if res.exec_time_ns is not None:
        print(f"HW exec time: {res.exec_time_ns} ns")
    outs = np.concatenate([res.results[c]["OUT"] for c in range(NCORES)],
                          axis=0)                   # [N, 128, S]
    # og col = (m, t, k): s = 128t + p, d = 2k + m
    full = (outs.reshape(N, 128, 2, NT, K2).transpose(0, 3, 1, 4, 2)
            .reshape(N, S, D).astype(np.float32))
    full = full.reshape(B, H, S, D).transpose(0, 2, 1, 3)
    return np.ascontiguousarray(full)
